# revision 1
# baseline (speedup 1.0000x reference)
"""ApproxNDCGLoss Trainium2 kernel (8 NeuronCores, data-parallel over graphs).

Math (per graph of G=128 candidates, labels binary):
  probs    = softmax(scores)            (no max-subtract: scores ~ N(0,1), fp32-safe)
  edcg     = sum_j probs_j * l_j * disc_j,   disc_j = 1/log2(j+2)
  idcg     = C(k), k = sum_j l_j, C = cumsum(disc)   (descending sort of binary
             labels == k ones first, so no sort needed)
  loss_g   = [k>0] * (1 - edcg/idcg);  loss = sum_g loss_g / B

On-chip pipeline per core (16384 graphs = 8 super-tiles of [128p x 2048f]):
  - gpsimd DMA loads scores f32->fp16 and labels int32->fp16 (cast in DMA)
  - DVE segmented reduce -> per-graph label count k
  - PE transposes each [128x128] sub-tile (regular fp16 matmul vs identity)
    into PSUM so candidates land on the partition axis
  - ACT computes eD = exp(s + ln disc) with per-partition bias (folds the DCG
    discount into the softmax exp); per-partition accumP via matmul:
       denom_g = sum_c eD * inv_disc   (lhsT=eD,  rhs=inv_disc column)
       num_g   = sum_c (eD .* L)       (lhsT=eDL, rhs=ones column)
    PE outputs go to per-graph columns of persistent PSUM compact buffers.
  - phase 2 (compact [128x128] per core): 1/C(k) via degree-8 polynomial in
    ln k (max rel err 2.5e-5), ndcg = num * poly / denom, masked sum, one
    f32 matmul to reduce over partitions, scalar DMA out.
Host: shard inputs (pure reshape/slice), sum the 8 partial scalars, / B.
`batch` is repeat(arange(B), G) by construction and is never read.
"""

import sys
from contextlib import ExitStack

import numpy as np

TRN_REPO = "/opt/trn_rl_repo"
if TRN_REPO not in sys.path:
    sys.path.insert(0, TRN_REPO)

import concourse.bass as bass
import concourse.mybir as mybir
import concourse.tile as tile
from concourse.masks import make_identity

B = 131072
G = 128
NCORES = 8
BPC = B // NCORES            # graphs per core
SUPERS = 8                   # super-tiles per core
FREE = 2048                  # free dim of a super-tile (16 graphs x 128 cand)
GPP = FREE // G              # graphs per partition per super-tile
COLS = BPC // 128            # compact columns per core (= SUPERS * GPP = 128)

F32 = mybir.dt.float32
F16 = mybir.dt.float16
I32 = mybir.dt.int32


def _fit_poly():
    """Degree-8 poly p(t) ~= 1/C(e^t), t = ln k, exact-ish at k = 1..128."""
    disc = 1.0 / np.log2(np.arange(1, G + 1, dtype=np.float64) + 1.0)
    C = np.cumsum(disc)
    k = np.arange(1, G + 1, dtype=np.float64)
    t = np.log(k)
    g = 1.0 / C
    w = 1.0 / g
    deg = 8
    for _ in range(40):
        cf = np.polyfit(t, g, deg, w=w)
        rel = (np.polyval(cf, t) - g) / g
        w = w * (1 + 3 * np.abs(rel) / np.abs(rel).max())
    return [float(c) for c in cf]


POLY = _fit_poly()


def _make_consts():
    pos = np.arange(1, G + 1, dtype=np.float64)
    invd = np.log2(pos + 1.0)                       # 1/disc
    invd16 = invd.astype(np.float16).astype(np.float32)
    # disc actually applied on-chip = 1/fp16(invd): makes denom reconstruction
    # (eD * invd16) exact in fp32, pushing all fp16 weight error into num
    lndisc = (-np.log(invd16.astype(np.float64))).astype(np.float32)
    consts = np.zeros((128, 4), dtype=np.float32)
    consts[:, 0] = lndisc
    consts[:, 1] = invd16
    return consts


def _split_drain_waits(nc, max_waits=1):
    """Workaround: this neuronxcc build rejects instructions carrying more
    than ~1 sem wait ("Too many sync wait commands"). Hoist excess waits
    onto standalone InstEventSemaphore instructions issued immediately
    before, on the same engine queue (in-order, so semantics unchanged)."""
    ctr = 0
    for f in nc.m.functions:
        for blk in f.blocks:
            new_list = []
            for inst in blk.instructions:
                si = inst.sync_info
                if (
                    si is not None
                    and si.on_wait
                    and len(si.on_wait) > max_waits
                    and not isinstance(inst, mybir.InstEventSemaphore)
                ):
                    keep = si.on_wait[-max_waits:]
                    for wt in si.on_wait[:-max_waits]:
                        ctr += 1
                        ev = mybir.InstEventSemaphore(
                            name=f"hoistwait-{ctr}",
                            ins=[],
                            outs=[],
                            sync_info=mybir.SyncInfo(on_wait=[wt], on_update=[]),
                        )
                        ev.engine = inst.engine
                        new_list.append(ev)
                    si.on_wait = keep
                new_list.append(inst)
            blk.instructions = new_list


def build_nc(repeats=1):
    """repeats>1 unrolls the main pipeline R times over the same data
    (identical results) — used only for device-time measurement."""
    AF = mybir.ActivationFunctionType
    ALU = mybir.AluOpType
    AX = mybir.AxisListType

    nc = bass.Bass("TRN2", target_bir_lowering=False, debug=False, num_devices=NCORES)
    scores_d = nc.dram_tensor("scores", [SUPERS, 128, FREE], F32, kind="ExternalInput").ap()
    labels_d = nc.dram_tensor("labels", [SUPERS, 128, FREE], I32, kind="ExternalInput").ap()
    consts_d = nc.dram_tensor("consts", [128, 4], F32, kind="ExternalInput").ap()
    out_d = nc.dram_tensor("out", [1, 1], F32, kind="ExternalOutput").ap()

    with tile.TileContext(nc) as tc:
        with ExitStack() as ctx:
            cpool = ctx.enter_context(tc.tile_pool(name="consts", bufs=1))
            ident = cpool.tile([128, 128], F16)
            make_identity(nc, ident[:])
            cvec = cpool.tile([128, 4], F32)
            nc.sync.dma_start(cvec[:], consts_d[:])
            lndisc = cvec[:, 0:1]
            invd16 = cpool.tile([128, 1], F16)
            nc.vector.tensor_copy(invd16[:], cvec[:, 1:2])
            ones16 = cpool.tile([128, 1], F16)
            nc.vector.memset(ones16[:], 1.0)
            onesf = cpool.tile([128, 1], F32)
            nc.gpsimd.memset(onesf[:], 1.0)

            kpool = ctx.enter_context(tc.tile_pool(name="ck", bufs=1))
            compact_k = kpool.tile([128, COLS], F32)
            pdp = ctx.enter_context(tc.tile_pool(name="cd", bufs=1, space="PSUM"))
            compact_d = pdp.tile([128, COLS], F32)
            pnp = ctx.enter_context(tc.tile_pool(name="cn", bufs=1, space="PSUM"))
            compact_n = pnp.tile([128, COLS], F32)

            # PSUM budget (8 banks): psS 2x1 + psL 2x2 + compact_d 1 +
            # compact_n 1 = 8; the final-scalar matmul reuses a psS slot.
            psS_pool = ctx.enter_context(tc.tile_pool(name="psS", bufs=2, space="PSUM"))
            psL_pool = ctx.enter_context(tc.tile_pool(name="psL", bufs=2, space="PSUM"))
            spool = ctx.enter_context(tc.tile_pool(name="s16", bufs=4))
            lpool = ctx.enter_context(tc.tile_pool(name="l16", bufs=4))
            epool = ctx.enter_context(tc.tile_pool(name="eD", bufs=4))
            e2pool = ctx.enter_context(tc.tile_pool(name="eDL", bufs=4))
            lcpool = ctx.enter_context(tc.tile_pool(name="lc16", bufs=3))
            khpool = ctx.enter_context(tc.tile_pool(name="kh", bufs=2))
            kh2pool = ctx.enter_context(tc.tile_pool(name="kh2", bufs=2))

            ph = ctx.enter_context(tc.tile_pool(name="ph", bufs=1))
            colparts = [ph.tile([128, 1], F32, name=f"cp{h}", tag=f"cp{h}") for h in range(2)]

            def phase2_half(h):
                lo, hi = h * (COLS // 2), (h + 1) * (COLS // 2)
                W = COLS // 2
                kc = ph.tile([128, W], F32, tag="p2kc")
                nc.vector.tensor_scalar_max(kc[:], compact_k[:, lo:hi], 1.0)
                tl = ph.tile([128, W], F32, tag="p2tl")
                nc.scalar.activation(tl[:], kc[:], AF.Ln)
                r = ph.tile([128, W], F32, tag="p2r")
                nc.vector.tensor_scalar_mul(r[:], tl[:], float(POLY[0]))
                for c in POLY[1:-1]:
                    nc.vector.scalar_tensor_tensor(
                        r[:], r[:], float(c), tl[:], op0=ALU.add, op1=ALU.mult
                    )
                w = ph.tile([128, W], F32, tag="p2w")
                nc.vector.scalar_tensor_tensor(
                    w[:], r[:], float(POLY[-1]), compact_n[:, lo:hi],
                    op0=ALU.add, op1=ALU.mult,
                )
                rd = ph.tile([128, W], F32, tag="p2rd")
                nc.vector.reciprocal(rd[:], compact_d[:, lo:hi])
                qt = ph.tile([128, W], F32, tag="p2qt")
                nc.vector.tensor_tensor(qt[:], w[:], rd[:], op=ALU.mult)
                vt = ph.tile([128, W], F32, tag="p2vt")
                nc.vector.tensor_scalar(vt[:], compact_k[:, lo:hi], 0.5, None, op0=ALU.is_ge)
                dlt = ph.tile([128, W], F32, tag="p2dlt")
                nc.vector.tensor_tensor(dlt[:], vt[:], qt[:], op=ALU.subtract)
                nc.vector.reduce_sum(colparts[h][:], dlt[:], axis=AX.X)

            for _rep in range(repeats):
                for s in range(SUPERS):
                    s16 = spool.tile([128, FREE], F16)
                    nc.gpsimd.dma_start(s16[:], scores_d[s])
                    l16 = lpool.tile([128, FREE], F16)
                    nc.gpsimd.dma_start(l16[:], labels_d[s])
                    # per-graph label count: two pairwise-halving adds (fp16
                    # SBUF tensor_tensor, 2x-mode eligible; values <= 4 exact
                    # in fp16), then segmented-reduce the quartered data
                    l3 = l16[:].rearrange("p (a b) -> p a b", a=GPP)
                    kh = khpool.tile([128, FREE // 2], F16)
                    kh3 = kh[:].rearrange("p (a b) -> p a b", a=GPP)
                    nc.vector.tensor_tensor(
                        kh3, l3[:, :, 0 : G // 2], l3[:, :, G // 2 : G], op=ALU.add
                    )
                    kh2 = kh2pool.tile([128, FREE // 4], F16)
                    kh23 = kh2[:].rearrange("p (a b) -> p a b", a=GPP)
                    nc.vector.tensor_tensor(
                        kh23, kh3[:, :, 0 : G // 4], kh3[:, :, G // 4 : G // 2],
                        op=ALU.add,
                    )
                    nc.vector.reduce_sum(
                        compact_k[:, s * GPP : (s + 1) * GPP], kh23, axis=AX.X
                    )
                    # process quads in pairs: psL/eD/eDL at [128,1024] grain
                    # to halve DVE per-instruction overhead on the mask-mult
                    for qp in range(2):
                        psL = psL_pool.tile([128, 1024], F32)
                        eD = epool.tile([128, 1024], F16)
                        for q2 in range(2):
                            q = qp * 2 + q2
                            psS = psS_pool.tile([128, 512], F32)
                            for i in range(4):
                                j = q * 4 + i
                                nc.tensor.matmul(
                                    psS[:, i * 128 : (i + 1) * 128],
                                    s16[:, j * 128 : (j + 1) * 128],
                                    ident[:],
                                    start=True,
                                    stop=True,
                                )
                            for i in range(4):
                                j = q * 4 + i
                                nc.tensor.matmul(
                                    psL[:, (q2 * 4 + i) * 128 : (q2 * 4 + i + 1) * 128],
                                    l16[:, j * 128 : (j + 1) * 128],
                                    ident[:],
                                    start=True,
                                    stop=True,
                                )
                            nc.scalar.activation(
                                eD[:, q2 * 512 : (q2 + 1) * 512],
                                psS[:],
                                AF.Exp,
                                bias=lndisc,
                                scale=1.0,
                            )
                        eDL = e2pool.tile([128, 1024], F16)
                        if qp == 0:
                            # ACT (slack) launders psL to SBUF fp16 so DVE's
                            # multiply runs in the 2x packed mode
                            lc16 = lcpool.tile([128, 1024], F16)
                            nc.scalar.activation(lc16[:], psL[:], AF.Copy)
                            nc.vector.tensor_tensor(
                                eDL[:], eD[:], lc16[:], op=ALU.mult
                            )
                        else:
                            # DVE reads psL directly (1x mode, f32 PSUM)
                            nc.vector.tensor_tensor(
                                eDL[:], eD[:], psL[:], op=ALU.mult
                            )
                        for i8 in range(8):
                            j = qp * 8 + i8
                            col = s * GPP + j
                            nc.tensor.matmul(
                                compact_d[:, col : col + 1],
                                eD[:, i8 * 128 : (i8 + 1) * 128],
                                invd16[:],
                                start=True,
                                stop=True,
                            )
                            nc.tensor.matmul(
                                compact_n[:, col : col + 1],
                                eDL[:, i8 * 128 : (i8 + 1) * 128],
                                ones16[:],
                                start=True,
                                stop=True,
                            )
                    if s == 3 and _rep == repeats - 1:
                        phase2_half(0)

            phase2_half(1)
            # ---- phase 2: per-graph scalars -> core scalar ----
            colr = ph.tile([128, 1], F32)
            nc.vector.tensor_tensor(colr[:], colparts[0][:], colparts[1][:], op=ALU.add)
            tot = psS_pool.tile([1, 1], F32, tag="psS")
            nc.tensor.matmul(tot[:], colr[:], onesf[:], start=True, stop=True)
            outsb = ph.tile([1, 1], F32)
            nc.scalar.activation(outsb[:], tot[:], AF.Copy)
            nc.sync.dma_start(out_d[:], outsb[:])

    _split_drain_waits(nc)
    return nc


_NC_CACHE = None


def get_nc():
    global _NC_CACHE
    if _NC_CACHE is None:
        _NC_CACHE = build_nc()
    return _NC_CACHE


def make_in_maps(scores, labels):
    scores_sh = np.ascontiguousarray(scores, dtype=np.float32).reshape(
        NCORES, SUPERS, 128, FREE
    )
    labels_sh = np.ascontiguousarray(labels, dtype=np.int32).reshape(
        NCORES, SUPERS, 128, FREE
    )
    consts = _make_consts()
    return [
        {"scores": scores_sh[c], "labels": labels_sh[c], "consts": consts}
        for c in range(NCORES)
    ]


_RUNNER_CACHE = None


def _get_runner():
    """Compile the NEFF + jitted shard_map executor once per process."""
    global _RUNNER_CACHE
    if _RUNNER_CACHE is not None:
        return _RUNNER_CACHE

    import jax
    from jax.sharding import Mesh, PartitionSpec, NamedSharding
    from jax.experimental.shard_map import shard_map
    from concourse import bass2jax

    nc = get_nc()
    bass2jax.install_neuronx_cc_hook()
    partition_name = nc.partition_id_tensor.name if nc.partition_id_tensor else None
    in_names, out_names, out_avals, zero_outs = [], [], [], []
    for alloc in nc.m.functions[0].allocations:
        if not isinstance(alloc, mybir.MemoryLocationSet):
            continue
        name = alloc.memorylocations[0].name
        if alloc.kind == "ExternalInput":
            if name != partition_name:
                in_names.append(name)
        elif alloc.kind == "ExternalOutput":
            shape = tuple(alloc.tensor_shape)
            dtype = mybir.dt.np(alloc.dtype)
            out_names.append(name)
            out_avals.append(jax.core.ShapedArray(shape, dtype))
            zero_outs.append(np.zeros(shape, dtype))
    n_params = len(in_names)
    n_outs = len(out_avals)
    all_in_names = list(in_names) + list(out_names)
    if partition_name is not None:
        all_in_names.append(partition_name)

    def _body(*args):
        operands = list(args)
        if partition_name is not None:
            operands.append(bass2jax.partition_id_tensor())
        return tuple(
            bass2jax._bass_exec_p.bind(
                *operands,
                out_avals=tuple(out_avals),
                in_names=tuple(all_in_names),
                out_names=tuple(out_names),
                lowering_input_output_aliases=(),
                sim_require_finite=True,
                sim_require_nnan=True,
                nc=nc,
            )
        )

    devices = jax.devices()[:NCORES]
    mesh = Mesh(np.asarray(devices), ("core",))
    sharded = jax.jit(
        shard_map(
            _body,
            mesh=mesh,
            in_specs=(PartitionSpec("core"),) * (n_params + n_outs),
            out_specs=(PartitionSpec("core"),) * n_outs,
            check_rep=False,
        ),
        keep_unused=True,
    )
    sharding = NamedSharding(mesh, PartitionSpec("core"))

    def run(in_maps):
        concat_in = [
            np.concatenate(
                [np.asarray(in_maps[c][nm]) for c in range(NCORES)], axis=0
            )
            for nm in in_names
        ]
        concat_zeros = [
            np.zeros((NCORES * z.shape[0], *z.shape[1:]), z.dtype) for z in zero_outs
        ]
        dev_in = [jax.device_put(a, sharding) for a in concat_in]
        dev_zeros = [jax.device_put(a, sharding) for a in concat_zeros]
        outs = sharded(*dev_in, *dev_zeros)
        outs = [np.asarray(o) for o in outs]
        return {
            nm: outs[i].reshape(NCORES, *out_avals[i].shape) for i, nm in enumerate(out_names)
        }

    _RUNNER_CACHE = run
    return run


def kernel(scores, labels, batch):
    run = _get_runner()
    in_maps = make_in_maps(scores, labels)
    outs = run(in_maps)
    total = float(outs["out"].sum())
    return np.float32(total / B)

